# Initial kernel scaffold
#
"""ConvNeXtV2 block (B=32, C=256, T=4096, K=9, H=512) on 8 trn2 cores.

Data-parallel over batch: 4 samples per core, no collectives.

v2 design notes (vs v0 baseline):
- x loaded ONCE per sample via SWDGE cast-DMA (f32 HBM -> bf16 SBUF,
  2 DMAs/sample) and kept resident for the residual -> HBM traffic drops
  from ~52MB to 33.6MB/core and DMA instruction count from ~335 to ~60.
- no gpsimd partition_broadcast / partition_all_reduce (measured ~100x
  slower on HW than the cost model): LN row stats are broadcast across
  partitions with K=1 ones-matmuls into PSUM, consumed directly by DVE.
- dwconv matmuls in bf16 (FWL-eligible) instead of fp32r.
- LN "+(-mu*r)" term folded into pw1 as a rank-1 matmul (W1s x nmr_row);
  only the multiplicative r needs a per-column broadcast.
- LN stat rows compacted [1,T]->[16,128] per T-half with one reshape DMA
  each, emitted mid-dwconv so the rep matmuls never stall.
- deferred-GRN software pipeline: iter s = dw+stats(s) | grn+pw2(s-1) |
  norm+pw1(s); the gelu/square ACT tail of pw1(s) overlaps dw(s+1).
- output written with 8 [128,1024] f32 DMAs/sample.
Host pre-folds ln_w/ln_b into pw1 and grn_beta into the pw2 bias.
Modeled (CoreSim) per-core time ~378us vs ~335us for v0; measured device
time ~0.13-0.5ms/pipeline vs ~5.5-8ms for v0 (gpsimd broadcasts + DMA
overheads dominate v0 on real HW).
"""

from contextlib import ExitStack

import ml_dtypes
import numpy as np

import concourse.bass as bass
import concourse.mybir as mybir
import concourse.tile as tile
from concourse import bacc
from concourse.bass_utils import run_bass_kernel_spmd

B, C, T, K, H = 32, 256, 4096, 9, 512
NCORES = 8
BL = B // NCORES          # samples per core
P = 128
NCC = C // P              # 2 channel chunks
NHC = H // P              # 4 hidden chunks
NBLK = T // 512           # 8 column blocks of 512
HALF = K // 2             # 4
TP = T + 2 * HALF         # padded row length
F32 = mybir.dt.float32
F32R = mybir.dt.float32r
BF16 = mybir.dt.bfloat16
I32 = mybir.dt.int32
BF = ml_dtypes.bfloat16
ALU = mybir.AluOpType
AF = mybir.ActivationFunctionType

_CACHE = {}
_REPEAT = 1    # timing-only knob: emit the whole pipeline N times in one NEFF
_PFX = [""]    # tile-name suffix per repeat (names must be unique)


def _rsqrt(nc, pool, v, pdim, n, tag):
    """Newton rsqrt on DVE for a small [pdim, n] f32 tile (avoids the ACT
    sqrt table set; gelu set stays resident)."""
    vi = pool.tile([pdim, n], I32, tag=f"{tag}_i", name=f"{tag}_i")
    nc.vector.tensor_scalar(
        out=vi, in0=v.bitcast(I32), scalar1=1, scalar2=None,
        op0=ALU.logical_shift_right,
    )
    nc.vector.tensor_scalar(out=vi, in0=vi, scalar1=0x5F3759DF, scalar2=-1,
                            op0=ALU.subtract, op1=ALU.mult)
    r = pool.tile([pdim, n], F32, tag=f"{tag}_r", name=f"{tag}_r")
    nc.vector.tensor_copy(out=r, in_=vi.bitcast(F32))
    h = pool.tile([pdim, n], F32, tag=f"{tag}_h", name=f"{tag}_h")
    for _ in range(3):
        nc.vector.tensor_mul(out=h, in0=r, in1=r)
        nc.vector.tensor_mul(out=h, in0=h, in1=v)
        nc.vector.tensor_scalar(
            out=h, in0=h, scalar1=-0.5, scalar2=1.5, op0=ALU.mult, op1=ALU.add
        )
        nc.vector.tensor_mul(out=r, in0=r, in1=h)
    return r


def _build():
    nc = bacc.Bacc(
        "TRN2", target_bir_lowering=False, debug=False, num_devices=NCORES
    )
    x_d = nc.dram_tensor("x", [BL, C, T], F32, kind="ExternalInput").ap()
    # all [128,n] constants byte-packed into one tensor -> one cold DMA
    CPB = 5756   # f32 block (120) + bf16 block (4352) + ones/w1s rows (1282)
    cpack_d = nc.dram_tensor("cpack", [P, CPB], mybir.dt.uint8,
                             kind="ExternalInput").ap()
    out_d = nc.dram_tensor("out", [BL, C, T], F32, kind="ExternalOutput").ap()

    with tile.TileContext(nc) as tc:
        with ExitStack() as ctx:
            _emit(ctx, tc, nc, x_d, out_d, cpack_d)
    nc.compile()
    return nc


def _emit(ctx, tc, nc, x_d, out_d, cpack_d):
    const = ctx.enter_context(tc.tile_pool(name="const", bufs=1))
    xb_p = ctx.enter_context(tc.tile_pool(name="xb", bufs=12))
    y_p = ctx.enter_context(tc.tile_pool(name="y", bufs=3))
    ysq_p = ctx.enter_context(tc.tile_pool(name="ysq", bufs=2))
    hid_p = ctx.enter_context(tc.tile_pool(name="hid", bufs=8))
    sm_p = ctx.enter_context(tc.tile_pool(name="sm", bufs=2))
    row_p = ctx.enter_context(tc.tile_pool(name="row", bufs=1))
    w2s_p = ctx.enter_context(tc.tile_pool(name="w2s", bufs=1))
    ob_p = ctx.enter_context(tc.tile_pool(name="ob", bufs=2))

    dw_ps = ctx.enter_context(tc.tile_pool(name="dwps", bufs=4, space="PSUM"))
    st_ps = ctx.enter_context(tc.tile_pool(name="stps", bufs=1, space="PSUM"))
    mm_ps = ctx.enter_context(tc.tile_pool(name="mmps", bufs=2, space="PSUM"))
    rep_ps = ctx.enter_context(tc.tile_pool(name="repps", bufs=1, space="PSUM"))

    # ---- constants: ONE packed DMA, then bitcast slices ----
    cp = const.tile([P, 5756], mybir.dt.uint8)
    nc.sync.dma_start(out=cp, in_=cpack_d)
    cpf = cp.bitcast(F32)            # [P, 1118] f32 view
    dww_s = cpf[:, 0:18]
    dwb_s = cpf[:, 18:20]
    b1f_s = cpf[:, 20:24]
    gam_s = cpf[:, 24:28]
    b2c_s = cpf[:, 28:30]
    cpb = cp.bitcast(BF16)           # [P, 2236] bf16 view
    ident_s = cpb[:, 60:60 + P]
    w1t_s = cpb[:, 188:188 + NCC * H]
    w2t_s = cpb[:, 1212:1212 + NHC * C]
    # build the 18 diagonal lhsT blocks on-chip (saves ~550KB of cold
    # const DMA inside the measured kernel span)
    diag_s = const.tile([P, K * NCC * P], BF16)
    for _idx in range(K * NCC):
        nc.vector.tensor_scalar(
            out=diag_s[:, _idx * P:(_idx + 1) * P], in0=ident_s,
            scalar1=dww_s[:, _idx:_idx + 1], scalar2=None, op0=ALU.mult)
    ones_col = cpb[:, 2236:2237]
    w1s_s = cpb[0:1, 2237:2237 + H]
    ones_row = cpb[0:1, 2749:2749 + P]

    xb = {}       # (s, cc) -> bf16 [P, TP] padded input
    y = {}        # (s, cc) -> bf16 [P, T]
    hid = {}      # (s, hc) -> bf16 [P, T]
    rows = {}     # s -> (r_row, nmr_row) bf16 [1, T]
    w2s = {}      # s -> scaled pw2 lhsT

    HT = T // 2            # 2048 t-columns per half-tile
    HW_ = HT + 2 * HALF    # 2056: half + halo on both sides

    def load(s):
        # two half-tiles per (s,cc): dwconv of the first T-half only waits
        # for the first 1MB of the cold cast-DMA, not the full 2MB
        for hh in range(2):        # half-outer: both cc first-halves load
            for cc in range(NCC):  # before any second half (stats need both)
                t = xb_p.tile([P, HW_], BF16, tag="xb",
                              name=f"xb_{s}_{cc}_{hh}{_PFX[0]}")
                xb[(s, cc, hh)] = t
                t0 = max(0, hh * HT - HALF)
                t1 = min(T, hh * HT + HT + HALF)
                j0 = t0 - (hh * HT - HALF)
                nc.gpsimd.dma_start(out=t[:, j0:j0 + (t1 - t0)],
                                    in_=x_d[s, cc * P:(cc + 1) * P, t0:t1])
                if hh == 0:
                    nc.vector.tensor_copy(
                        out=t[:, 0:HALF],
                        in_=t[:, HALF:HALF + 1].to_broadcast((P, HALF)))
                else:
                    nc.vector.tensor_copy(
                        out=t[:, HW_ - HALF:HW_],
                        in_=t[:, HW_ - HALF - 1:HW_ - HALF].to_broadcast(
                            (P, HALF)))

    def ln_half(s, hf, s_row, q_row, r_row, nmr_row):
        # LN math for one T-half on compact [16,128] tiles; emitted as soon
        # as that half's stats are drained so the rep matmuls never stall.
        HL = T // 2
        s_c = sm_p.tile([16, P], BF16, tag=f"s_c{hf}", name=f"s_c_{s}_{hf}{_PFX[0]}")
        q_c = sm_p.tile([16, P], BF16, tag=f"q_c{hf}", name=f"q_c_{s}_{hf}{_PFX[0]}")
        nc.sync.dma_start(out=s_c, in_=s_row[:, hf * HL:(hf + 1) * HL])
        nc.sync.dma_start(out=q_c, in_=q_row[:, hf * HL:(hf + 1) * HL])
        mu = sm_p.tile([16, P], F32, tag=f"mu{hf}")
        nc.vector.tensor_scalar(out=mu, in0=s_c, scalar1=1.0 / C, scalar2=None,
                                op0=ALU.mult)
        var = sm_p.tile([16, P], F32, tag=f"var{hf}")
        nc.vector.tensor_mul(out=var, in0=mu, in1=mu)
        nc.vector.scalar_tensor_tensor(
            out=var, in0=q_c, scalar=1.0 / C, in1=var,
            op0=ALU.mult, op1=ALU.subtract)
        nc.vector.tensor_scalar(out=var, in0=var, scalar1=1e-5, scalar2=None,
                                op0=ALU.add)
        r = _rsqrt(nc, sm_p, var, 16, P, f"rs{hf}")
        nmr = sm_p.tile([16, P], F32, tag=f"nmr{hf}")
        nc.vector.scalar_tensor_tensor(out=nmr, in0=mu, scalar=-1.0, in1=r,
                                       op0=ALU.mult, op1=ALU.mult)
        r_bf = sm_p.tile([16, P], BF16, tag=f"r_bf{hf}")
        nc.vector.tensor_copy(out=r_bf, in_=r)
        nmr_bf = sm_p.tile([16, P], BF16, tag=f"nmr_bf{hf}")
        nc.vector.tensor_copy(out=nmr_bf, in_=nmr)
        nc.sync.dma_start(out=r_row[:, hf * HL:(hf + 1) * HL], in_=r_bf)
        nc.sync.dma_start(out=nmr_row[:, hf * HL:(hf + 1) * HL], in_=nmr_bf)

    def dw_stats(s):
        for cc in range(NCC):
            y[(s, cc)] = y_p.tile([P, T], BF16, tag="y", name=f"y_{s}_{cc}{_PFX[0]}")
        s_row = row_p.tile([1, T], BF16, tag="s_row", name=f"s_row_{s}{_PFX[0]}")
        q_row = row_p.tile([1, T], BF16, tag="q_row", name=f"q_row_{s}{_PFX[0]}")
        r_row = row_p.tile([1, T], BF16, tag="r_row", name=f"r_row_{s}{_PFX[0]}")
        nmr_row = row_p.tile([1, T], BF16, tag="nmr_row", name=f"nmr_row_{s}{_PFX[0]}")
        rows[s] = (r_row, nmr_row)
        for blk in range(NBLK):
            lo = blk * 512
            ysqs = []
            for cc in range(NCC):
                ps = dw_ps.tile([P, 512], F32, tag="dwps")
                xt = xb[(s, cc, blk // 4)]
                loh = 512 * (blk % 4)
                for k in range(K):
                    nc.tensor.matmul(
                        ps,
                        lhsT=diag_s[:, (k * NCC + cc) * P:(k * NCC + cc + 1) * P],
                        rhs=xt[:, loh + k:loh + k + 512],
                        start=(k == 0), stop=(k == K - 1),
                    )
                ysl = y[(s, cc)][:, lo:lo + 512]
                # drain psum + dw bias -> y bf16 (DVE; anything feeding the
                # stats matmuls must stay off ACT's gelu backlog)
                nc.vector.tensor_scalar(out=ysl, in0=ps,
                                        scalar1=dwb_s[:, cc:cc + 1],
                                        scalar2=None, op0=ALU.add)
                ysq = ysq_p.tile([P, 512], BF16, tag="ysq")
                nc.vector.tensor_mul(out=ysq, in0=ysl, in1=ysl)
                ysqs.append(ysq)
            # S and Q chains in different PE column groups -> they run
            # concurrently (one [64,512] psum tile, S@p0, Q@p32)
            st2 = st_ps.tile([64, 512], F32, tag="stps",
                             name=f"st2_{s}_{blk}{_PFX[0]}")
            for cc in range(NCC):
                nc.tensor.matmul(st2[0:1, :], lhsT=ones_col,
                                 rhs=y[(s, cc)][:, lo:lo + 512],
                                 start=(cc == 0), stop=(cc == NCC - 1),
                                 tile_position=(0, 0), skip_group_check=True)
                nc.tensor.matmul(st2[32:33, :], lhsT=ones_col, rhs=ysqs[cc],
                                 start=(cc == 0), stop=(cc == NCC - 1),
                                 tile_position=(0, 32), skip_group_check=True)
            nc.vector.tensor_copy(out=s_row[:, lo:lo + 512], in_=st2[0:1, :])
            nc.vector.tensor_copy(out=q_row[:, lo:lo + 512], in_=st2[32:33, :])
            if blk == NBLK // 2 - 1:
                ln_half(s, 0, s_row, q_row, r_row, nmr_row)
        ln_half(s, 1, s_row, q_row, r_row, nmr_row)

    def norm(s):
        # y *= r (per-column, via PSUM-resident broadcast); the "+nmr" term
        # is folded into pw1 as a rank-1 matmul (W1s ⊗ nmr_row).
        r_row, _ = rows[s]
        for blk in range(NBLK):
            lo = blk * 512
            r_ps = rep_ps.tile([P, 512], F32, tag="repps")
            nc.tensor.matmul(r_ps, lhsT=ones_row, rhs=r_row[:, lo:lo + 512],
                             start=True, stop=True)
            for cc in range(NCC):
                ysl = y[(s, cc)][:, lo:lo + 512]
                nc.vector.tensor_mul(out=ysl, in0=ysl, in1=r_ps)

    gx2s = {}

    def pw1(s):
        for hc in range(NHC):
            hid[(s, hc)] = hid_p.tile([P, T], BF16, tag="hid",
                                      name=f"hid_{s}_{hc}{_PFX[0]}")
            nmr_row = rows[s][1]
            for blk in range(NBLK):
                ps = mm_ps.tile([P, 512], F32, tag="mmps")
                for cc in range(NCC):
                    nc.tensor.matmul(
                        ps, lhsT=w1t_s[:, cc * H + hc * P:cc * H + (hc + 1) * P],
                        rhs=y[(s, cc)][:, blk * 512:(blk + 1) * 512],
                        start=(cc == 0), stop=False)
                # rank-1: += W1s[hc-chunk] ⊗ nmr_row  (the LN "-mu*r" term)
                nc.tensor.matmul(
                    ps, lhsT=w1s_s[:, hc * P:(hc + 1) * P],
                    rhs=nmr_row[:, blk * 512:(blk + 1) * 512],
                    start=False, stop=True)
                nc.scalar.activation(
                    out=hid[(s, hc)][:, blk * 512:(blk + 1) * 512],
                    in_=ps, func=AF.Gelu, bias=b1f_s[:, hc:hc + 1], scale=1.0)
        # GRN square+accum; y(s,0/1) are dead after pw1 reads -> reuse as
        # scratch. For the last sample (nothing left to overlap), split the
        # squares across ACT and DVE to halve the tail stall.
        gx2 = sm_p.tile([P, NHC], F32, tag="gx2", name=f"gx2_{s}{_PFX[0]}")
        gx2s[s] = gx2
        for hc in range(NHC):
            if s == BL - 1 and hc >= 2:
                sq = y[(s, 1)]
                nc.vector.tensor_mul(out=sq, in0=hid[(s, hc)], in1=hid[(s, hc)])
                nc.vector.tensor_reduce(out=gx2[:, hc:hc + 1], in_=sq,
                                        axis=mybir.AxisListType.X, op=ALU.add)
            else:
                nc.scalar.activation(out=y[(s, 0)], in_=hid[(s, hc)],
                                     func=AF.Square, accum_out=gx2[:, hc:hc + 1])

    def grn(s):
        gx2 = gx2s[s]
        gx2f = sm_p.tile([P, NHC], F32, tag="gx2f")
        nc.vector.tensor_scalar(out=gx2f, in0=gx2, scalar1=1e-30, scalar2=None,
                                op0=ALU.add)
        rg = _rsqrt(nc, sm_p, gx2f, P, NHC, "rg")
        gx = sm_p.tile([P, NHC], F32, tag="gx")
        nc.vector.tensor_mul(out=gx, in0=gx2f, in1=rg)      # gx = sqrt(gx2)
        gx_bf = sm_p.tile([P, NHC], BF16, tag="gx_bf")
        nc.vector.tensor_copy(out=gx_bf, in_=gx)
        # mean over all H=512 channels: ones-matmul -> [1,4] -> reduce
        gt_ps = st_ps.tile([1, NHC], F32, tag="stps", name=f"gt_{s}{_PFX[0]}")
        nc.tensor.matmul(gt_ps, lhsT=ones_col, rhs=gx_bf,
                         start=True, stop=True)
        g_row = sm_p.tile([1, NHC], F32, tag="g_row")
        nc.vector.tensor_copy(out=g_row, in_=gt_ps)
        tot = sm_p.tile([1, 1], F32, tag="tot")
        nc.vector.tensor_reduce(out=tot, in_=g_row, axis=mybir.AxisListType.X,
                                op=ALU.add)
        nc.vector.tensor_scalar(out=tot, in0=tot, scalar1=1.0 / H,
                                scalar2=1e-6, op0=ALU.mult, op1=ALU.add)
        rm_row = sm_p.tile([1, 1], F32, tag="rm_row")
        nc.vector.reciprocal(out=rm_row, in_=tot)
        rm_bf = sm_p.tile([1, 1], BF16, tag="rm_bf")
        nc.vector.tensor_copy(out=rm_bf, in_=rm_row)
        rm_ps = st_ps.tile([P, 1], F32, tag="stps", name=f"rm_{s}{_PFX[0]}")
        nc.tensor.matmul(rm_ps, lhsT=ones_row, rhs=rm_bf,
                         start=True, stop=True)
        rm = sm_p.tile([P, 1], F32, tag="rm")
        nc.vector.tensor_copy(out=rm, in_=rm_ps)
        a = sm_p.tile([P, NHC], F32, tag="a")
        nc.vector.tensor_scalar(out=a, in0=gx, scalar1=rm, scalar2=None,
                                op0=ALU.mult)
        nc.vector.scalar_tensor_tensor(out=a, in0=a, scalar=1.0, in1=gam_s,
                                       op0=ALU.bypass, op1=ALU.mult)
        nc.vector.tensor_scalar(out=a, in0=a, scalar1=1.0, scalar2=None,
                                op0=ALU.add)
        w2s[s] = w2s_p.tile([P, NHC * C], BF16, tag="w2s", name=f"w2s_{s}{_PFX[0]}")
        for hc in range(NHC):
            nc.vector.tensor_scalar(
                out=w2s[s][:, hc * C:(hc + 1) * C],
                in0=w2t_s[:, hc * C:(hc + 1) * C],
                scalar1=a[:, hc:hc + 1], scalar2=None, op0=ALU.mult)

    def pw2_merge(s):
        for cc in range(NCC):
            for ob_i in range(4):          # four [P, 1024] output tiles per cc
                ob = ob_p.tile([P, 1024], F32, tag="ob")
                for sub in range(2):
                    blk = ob_i * 2 + sub
                    lo = blk * 512
                    ps = mm_ps.tile([P, 512], F32, tag="mmps")
                    for hc in range(NHC):
                        nc.tensor.matmul(
                            ps,
                            lhsT=w2s[s][:, hc * C + cc * P:hc * C + (cc + 1) * P],
                            rhs=hid[(s, hc)][:, lo:lo + 512],
                            start=(hc == 0), stop=(hc == NHC - 1))
                    # out = psum + bias2 + x  (one DVE op, x read back as bf16)
                    nc.vector.scalar_tensor_tensor(
                        out=ob[:, sub * 512:(sub + 1) * 512], in0=ps,
                        scalar=b2c_s[:, cc:cc + 1],
                        in1=xb[(s, cc, blk // 4)][:, HALF + 512 * (blk % 4):
                                                  HALF + 512 * (blk % 4) + 512],
                        op0=ALU.add, op1=ALU.add)
                nc.sync.dma_start(
                    out=out_d[s, cc * P:(cc + 1) * P,
                              ob_i * 1024:(ob_i + 1) * 1024],
                    in_=ob)

    # deferred-GRN pipeline: iter s runs dw+stats(s) / grn+pw2(s-1) /
    # norm+pw1(s); gelu+square ACT tails of pw1(s) overlap dw(s+1), giving
    # the GRN chain a full iteration of slack before pw2(s) needs w2s.
    for rp in range(_REPEAT):
        _PFX[0] = f"_rp{rp}" if _REPEAT > 1 else ""
        load(0)
        for s in range(BL):
            if s + 1 < BL:
                load(s + 1)
            dw_stats(s)
            if s >= 1:
                grn(s - 1)
                pw2_merge(s - 1)
            norm(s)
            pw1(s)
        grn(BL - 1)
        pw2_merge(BL - 1)


def _prep_inputs(inputs):
    x = np.ascontiguousarray(np.asarray(inputs["x"], np.float32))
    dw_w = np.asarray(inputs["dw_w"], np.float32)      # (C,1,K)
    dw_b = np.asarray(inputs["dw_b"], np.float32)
    ln_w = np.asarray(inputs["ln_w"], np.float32)
    ln_b = np.asarray(inputs["ln_b"], np.float32)
    pw1_w = np.asarray(inputs["pw1_w"], np.float32)    # (H,C)
    pw1_b = np.asarray(inputs["pw1_b"], np.float32)
    gg = np.asarray(inputs["grn_gamma"], np.float32)
    gb = np.asarray(inputs["grn_beta"], np.float32)
    pw2_w = np.asarray(inputs["pw2_w"], np.float32)    # (C,H)
    pw2_b = np.asarray(inputs["pw2_b"], np.float32)

    ident = np.eye(P, dtype=BF)
    dww = np.zeros((P, K * NCC), np.float32)
    for k in range(K):
        for cc in range(NCC):
            dww[:, k * NCC + cc] = dw_w[cc * P:(cc + 1) * P, 0, k]
    dwb = dw_b.reshape(NCC, P).T.copy()

    w1f = pw1_w * ln_w[None, :]                        # (H,C)
    w1t = np.zeros((P, NCC * H), BF)
    for cc in range(NCC):
        for hc in range(NHC):
            w1t[:, cc * H + hc * P:cc * H + (hc + 1) * P] = \
                w1f[hc * P:(hc + 1) * P, cc * P:(cc + 1) * P].T.astype(BF)
    b1f = (pw1_b + pw1_w @ ln_b).reshape(NHC, P).T.copy()
    w1s = w1f.sum(axis=1).astype(BF).reshape(1, H)

    w2t = np.zeros((P, NHC * C), BF)
    for hc in range(NHC):
        w2t[:, hc * C:(hc + 1) * C] = \
            pw2_w[:, hc * P:(hc + 1) * P].T.astype(BF)
    gam = gg.reshape(NHC, P).T.copy()
    b2c = (pw2_b + pw2_w @ gb).reshape(NCC, P).T.copy()

    onescol = np.ones((P, 1), BF)
    w1s_blk = np.zeros((P, H), BF)
    w1s_blk[0, :] = w1s[0, :]
    onesrow_blk = np.zeros((P, P), BF)
    onesrow_blk[0, :] = 1.0
    cpack = np.concatenate([
        dww.view(np.uint8), dwb.view(np.uint8), b1f.view(np.uint8),
        gam.view(np.uint8), b2c.view(np.uint8), ident.view(np.uint8),
        w1t.view(np.uint8), w2t.view(np.uint8), onescol.view(np.uint8),
        w1s_blk.view(np.uint8), onesrow_blk.view(np.uint8),
        np.zeros((P, 2), np.uint8)], axis=1)
    assert cpack.shape == (P, 5756), cpack.shape
    common = {"cpack": np.ascontiguousarray(cpack)}
    in_maps = []
    for i in range(NCORES):
        m = dict(common)
        m["x"] = np.ascontiguousarray(x[i * BL:(i + 1) * BL])
        in_maps.append(m)
    return in_maps


def kernel(**inputs):
    if "nc" not in _CACHE:
        _CACHE["nc"] = _build()
    nc = _CACHE["nc"]
    in_maps = _prep_inputs(inputs)
    res = run_bass_kernel_spmd(nc, in_maps, core_ids=list(range(NCORES)),
                               **_CACHE.get("run_kwargs", {}))
    _CACHE["last_result"] = res
    out = np.concatenate([res.results[i]["out"] for i in range(NCORES)], axis=0)
    return out



# revision 1
# speedup vs baseline: 1.1817x; 1.1817x over previous
"""ConvNeXtV2 block (B=32, C=256, T=4096, K=9, H=512) on 8 trn2 cores.

Data-parallel over batch: 4 samples per core, no collectives.

v2 design notes (vs v0 baseline):
- x loaded ONCE per sample via SWDGE cast-DMA (f32 HBM -> bf16 SBUF,
  2 DMAs/sample) and kept resident for the residual -> HBM traffic drops
  from ~52MB to 33.6MB/core and DMA instruction count from ~335 to ~60.
- no gpsimd partition_broadcast / partition_all_reduce (measured ~100x
  slower on HW than the cost model): LN row stats are broadcast across
  partitions with K=1 ones-matmuls into PSUM, consumed directly by DVE.
- dwconv matmuls in bf16 (FWL-eligible) instead of fp32r.
- LN "+(-mu*r)" term folded into pw1 as a rank-1 matmul (W1s x nmr_row);
  only the multiplicative r needs a per-column broadcast.
- LN stat rows compacted [1,T]->[16,128] per T-half with one reshape DMA
  each, emitted mid-dwconv so the rep matmuls never stall.
- deferred-GRN software pipeline: iter s = dw+stats(s) | grn+pw2(s-1) |
  norm+pw1(s); the gelu/square ACT tail of pw1(s) overlaps dw(s+1).
- output written with 8 [128,1024] f32 DMAs/sample.
Host pre-folds ln_w/ln_b into pw1 and grn_beta into the pw2 bias.
Modeled (CoreSim) per-core time ~378us vs ~335us for v0; measured device
time ~0.13-0.5ms/pipeline vs ~5.5-8ms for v0 (gpsimd broadcasts + DMA
overheads dominate v0 on real HW).
"""

from contextlib import ExitStack

import ml_dtypes
import numpy as np

import concourse.bass as bass
import concourse.mybir as mybir
import concourse.tile as tile
from concourse import bacc
from concourse.bass_utils import run_bass_kernel_spmd

B, C, T, K, H = 32, 256, 4096, 9, 512
NCORES = 8
BL = B // NCORES          # samples per core
P = 128
NCC = C // P              # 2 channel chunks
NHC = H // P              # 4 hidden chunks
NBLK = T // 512           # 8 column blocks of 512
HALF = K // 2             # 4
TP = T + 2 * HALF         # padded row length
F32 = mybir.dt.float32
F32R = mybir.dt.float32r
BF16 = mybir.dt.bfloat16
I32 = mybir.dt.int32
BF = ml_dtypes.bfloat16
ALU = mybir.AluOpType
AF = mybir.ActivationFunctionType

_CACHE = {}
_REPEAT = 1    # timing-only knob: emit the whole pipeline N times in one NEFF
_PFX = [""]    # tile-name suffix per repeat (names must be unique)


def _rsqrt(nc, pool, v, pdim, n, tag):
    """Newton rsqrt on DVE for a small [pdim, n] f32 tile (avoids the ACT
    sqrt table set; gelu set stays resident)."""
    vi = pool.tile([pdim, n], I32, tag=f"{tag}_i", name=f"{tag}_i")
    nc.vector.tensor_scalar(
        out=vi, in0=v.bitcast(I32), scalar1=1, scalar2=None,
        op0=ALU.logical_shift_right,
    )
    nc.vector.tensor_scalar(out=vi, in0=vi, scalar1=0x5F3759DF, scalar2=-1,
                            op0=ALU.subtract, op1=ALU.mult)
    r = pool.tile([pdim, n], F32, tag=f"{tag}_r", name=f"{tag}_r")
    nc.vector.tensor_copy(out=r, in_=vi.bitcast(F32))
    h = pool.tile([pdim, n], F32, tag=f"{tag}_h", name=f"{tag}_h")
    for _ in range(3):
        nc.vector.tensor_mul(out=h, in0=r, in1=r)
        nc.vector.tensor_mul(out=h, in0=h, in1=v)
        nc.vector.tensor_scalar(
            out=h, in0=h, scalar1=-0.5, scalar2=1.5, op0=ALU.mult, op1=ALU.add
        )
        nc.vector.tensor_mul(out=r, in0=r, in1=h)
    return r


def _build():
    nc = bacc.Bacc(
        "TRN2", target_bir_lowering=False, debug=False, num_devices=NCORES
    )
    x_d = nc.dram_tensor("x", [BL, C, T], F32, kind="ExternalInput").ap()
    # all [128,n] constants byte-packed into one tensor -> one cold DMA
    CPB = 5756   # f32 block (120) + bf16 block (4352) + ones/w1s rows (1282)
    cpack_d = nc.dram_tensor("cpack", [P, CPB], mybir.dt.uint8,
                             kind="ExternalInput").ap()
    out_d = nc.dram_tensor("out", [BL, C, T], F32, kind="ExternalOutput").ap()

    with tile.TileContext(nc) as tc:
        with ExitStack() as ctx:
            _emit(ctx, tc, nc, x_d, out_d, cpack_d)
    nc.compile()
    return nc


def _emit(ctx, tc, nc, x_d, out_d, cpack_d):
    const = ctx.enter_context(tc.tile_pool(name="const", bufs=1))
    xb_p = ctx.enter_context(tc.tile_pool(name="xb", bufs=12))
    y_p = ctx.enter_context(tc.tile_pool(name="y", bufs=3))
    ysq_p = ctx.enter_context(tc.tile_pool(name="ysq", bufs=2))
    hid_p = ctx.enter_context(tc.tile_pool(name="hid", bufs=8))
    sm_p = ctx.enter_context(tc.tile_pool(name="sm", bufs=2))
    row_p = ctx.enter_context(tc.tile_pool(name="row", bufs=1))
    w2s_p = ctx.enter_context(tc.tile_pool(name="w2s", bufs=1))
    ob_p = ctx.enter_context(tc.tile_pool(name="ob", bufs=2))

    dw_ps = ctx.enter_context(tc.tile_pool(name="dwps", bufs=4, space="PSUM"))
    st_ps = ctx.enter_context(tc.tile_pool(name="stps", bufs=1, space="PSUM"))
    mm_ps = ctx.enter_context(tc.tile_pool(name="mmps", bufs=2, space="PSUM"))
    rep_ps = ctx.enter_context(tc.tile_pool(name="repps", bufs=1, space="PSUM"))

    # ---- constants: ONE packed DMA, then bitcast slices ----
    cp = const.tile([P, 5756], mybir.dt.uint8)
    nc.sync.dma_start(out=cp, in_=cpack_d)
    cpf = cp.bitcast(F32)            # [P, 1118] f32 view
    dww_s = cpf[:, 0:18]
    dwb_s = cpf[:, 18:20]
    b1f_s = cpf[:, 20:24]
    gam_s = cpf[:, 24:28]
    b2c_s = cpf[:, 28:30]
    cpb = cp.bitcast(BF16)           # [P, 2236] bf16 view
    ident_s = cpb[:, 60:60 + P]
    w1t_s = cpb[:, 188:188 + NCC * H]
    w2t_s = cpb[:, 1212:1212 + NHC * C]
    # build the 18 diagonal lhsT blocks on-chip (saves ~550KB of cold
    # const DMA inside the measured kernel span)
    diag_s = const.tile([P, K * NCC * P], BF16)
    for _idx in range(K * NCC):
        nc.vector.tensor_scalar(
            out=diag_s[:, _idx * P:(_idx + 1) * P], in0=ident_s,
            scalar1=dww_s[:, _idx:_idx + 1], scalar2=None, op0=ALU.mult)
    ones_col = cpb[:, 2236:2237]
    w1s_s = cpb[0:1, 2237:2237 + H]
    ones_row = cpb[0:1, 2749:2749 + P]

    xb = {}       # (s, cc) -> bf16 [P, TP] padded input
    y = {}        # (s, cc) -> bf16 [P, T]
    hid = {}      # (s, hc) -> bf16 [P, T]
    rows = {}     # s -> (r_row, nmr_row) bf16 [1, T]
    w2s = {}      # s -> scaled pw2 lhsT

    HT = T // 2            # 2048 t-columns per half-tile
    HW_ = HT + 2 * HALF    # 2056: half + halo on both sides

    def load(s):
        # two half-tiles per (s,cc): dwconv of the first T-half only waits
        # for the first 1MB of the cold cast-DMA, not the full 2MB
        for hh in range(2):        # half-outer: both cc first-halves load
            for cc in range(NCC):  # before any second half (stats need both)
                t = xb_p.tile([P, HW_], BF16, tag="xb",
                              name=f"xb_{s}_{cc}_{hh}{_PFX[0]}")
                xb[(s, cc, hh)] = t
                t0 = max(0, hh * HT - HALF)
                t1 = min(T, hh * HT + HT + HALF)
                j0 = t0 - (hh * HT - HALF)
                nc.gpsimd.dma_start(out=t[:, j0:j0 + (t1 - t0)],
                                    in_=x_d[s, cc * P:(cc + 1) * P, t0:t1])
                if hh == 0:
                    nc.vector.tensor_copy(
                        out=t[:, 0:HALF],
                        in_=t[:, HALF:HALF + 1].to_broadcast((P, HALF)))
                else:
                    nc.vector.tensor_copy(
                        out=t[:, HW_ - HALF:HW_],
                        in_=t[:, HW_ - HALF - 1:HW_ - HALF].to_broadcast(
                            (P, HALF)))

    def ln_half(s, hf, s_row, q_row, r_row, nmr_row):
        # LN math for one T-half on compact [16,128] tiles; emitted as soon
        # as that half's stats are drained so the rep matmuls never stall.
        HL = T // 2
        s_c = sm_p.tile([16, P], BF16, tag=f"s_c{hf}", name=f"s_c_{s}_{hf}{_PFX[0]}")
        q_c = sm_p.tile([16, P], BF16, tag=f"q_c{hf}", name=f"q_c_{s}_{hf}{_PFX[0]}")
        nc.sync.dma_start(out=s_c, in_=s_row[:, hf * HL:(hf + 1) * HL])
        nc.sync.dma_start(out=q_c, in_=q_row[:, hf * HL:(hf + 1) * HL])
        mu = sm_p.tile([16, P], F32, tag=f"mu{hf}")
        nc.vector.tensor_scalar(out=mu, in0=s_c, scalar1=1.0 / C, scalar2=None,
                                op0=ALU.mult)
        var = sm_p.tile([16, P], F32, tag=f"var{hf}")
        nc.vector.tensor_mul(out=var, in0=mu, in1=mu)
        nc.vector.scalar_tensor_tensor(
            out=var, in0=q_c, scalar=1.0 / C, in1=var,
            op0=ALU.mult, op1=ALU.subtract)
        nc.vector.tensor_scalar(out=var, in0=var, scalar1=1e-5, scalar2=None,
                                op0=ALU.add)
        r = _rsqrt(nc, sm_p, var, 16, P, f"rs{hf}")
        nmr = sm_p.tile([16, P], F32, tag=f"nmr{hf}")
        nc.vector.scalar_tensor_tensor(out=nmr, in0=mu, scalar=-1.0, in1=r,
                                       op0=ALU.mult, op1=ALU.mult)
        r_bf = sm_p.tile([16, P], BF16, tag=f"r_bf{hf}")
        nc.vector.tensor_copy(out=r_bf, in_=r)
        nmr_bf = sm_p.tile([16, P], BF16, tag=f"nmr_bf{hf}")
        nc.vector.tensor_copy(out=nmr_bf, in_=nmr)
        nc.sync.dma_start(out=r_row[:, hf * HL:(hf + 1) * HL], in_=r_bf)
        nc.sync.dma_start(out=nmr_row[:, hf * HL:(hf + 1) * HL], in_=nmr_bf)

    def dw_stats(s):
        for cc in range(NCC):
            y[(s, cc)] = y_p.tile([P, T], BF16, tag="y", name=f"y_{s}_{cc}{_PFX[0]}")
        s_row = row_p.tile([1, T], BF16, tag="s_row", name=f"s_row_{s}{_PFX[0]}")
        q_row = row_p.tile([1, T], BF16, tag="q_row", name=f"q_row_{s}{_PFX[0]}")
        r_row = row_p.tile([1, T], BF16, tag="r_row", name=f"r_row_{s}{_PFX[0]}")
        nmr_row = row_p.tile([1, T], BF16, tag="nmr_row", name=f"nmr_row_{s}{_PFX[0]}")
        rows[s] = (r_row, nmr_row)
        for blk in range(NBLK):
            lo = blk * 512
            ysqs = []
            for cc in range(NCC):
                ps = dw_ps.tile([P, 512], F32, tag="dwps")
                xt = xb[(s, cc, blk // 4)]
                loh = 512 * (blk % 4)
                for k in range(K):
                    nc.tensor.matmul(
                        ps,
                        lhsT=diag_s[:, (k * NCC + cc) * P:(k * NCC + cc + 1) * P],
                        rhs=xt[:, loh + k:loh + k + 512],
                        start=(k == 0), stop=(k == K - 1),
                    )
                ysl = y[(s, cc)][:, lo:lo + 512]
                # drain psum + dw bias -> y bf16 (DVE; anything feeding the
                # stats matmuls must stay off ACT's gelu backlog)
                nc.vector.tensor_scalar(out=ysl, in0=ps,
                                        scalar1=dwb_s[:, cc:cc + 1],
                                        scalar2=None, op0=ALU.add)
                ysq = ysq_p.tile([P, 512], BF16, tag="ysq")
                nc.vector.tensor_mul(out=ysq, in0=ysl, in1=ysl)
                ysqs.append(ysq)
            # S and Q chains in different PE column groups -> they run
            # concurrently (one [64,512] psum tile, S@p0, Q@p32)
            st2 = st_ps.tile([64, 512], F32, tag="stps",
                             name=f"st2_{s}_{blk}{_PFX[0]}")
            for cc in range(NCC):
                nc.tensor.matmul(st2[0:1, :], lhsT=ones_col,
                                 rhs=y[(s, cc)][:, lo:lo + 512],
                                 start=(cc == 0), stop=(cc == NCC - 1),
                                 tile_position=(0, 0), skip_group_check=True)
                nc.tensor.matmul(st2[32:33, :], lhsT=ones_col, rhs=ysqs[cc],
                                 start=(cc == 0), stop=(cc == NCC - 1),
                                 tile_position=(0, 32), skip_group_check=True)
            nc.vector.tensor_copy(out=s_row[:, lo:lo + 512], in_=st2[0:1, :])
            nc.vector.tensor_copy(out=q_row[:, lo:lo + 512], in_=st2[32:33, :])
            if blk == NBLK // 2 - 1:
                ln_half(s, 0, s_row, q_row, r_row, nmr_row)
        ln_half(s, 1, s_row, q_row, r_row, nmr_row)

    def norm(s):
        # y *= r (per-column, via PSUM-resident broadcast); the "+nmr" term
        # is folded into pw1 as a rank-1 matmul (W1s ⊗ nmr_row).
        r_row, _ = rows[s]
        for blk in range(NBLK):
            lo = blk * 512
            r_ps = rep_ps.tile([P, 512], F32, tag="repps")
            nc.tensor.matmul(r_ps, lhsT=ones_row, rhs=r_row[:, lo:lo + 512],
                             start=True, stop=True)
            for cc in range(NCC):
                ysl = y[(s, cc)][:, lo:lo + 512]
                nc.vector.tensor_mul(out=ysl, in0=ysl, in1=r_ps)

    gx2s = {}

    def pw1(s):
        for hc in range(NHC):
            hid[(s, hc)] = hid_p.tile([P, T], BF16, tag="hid",
                                      name=f"hid_{s}_{hc}{_PFX[0]}")
            nmr_row = rows[s][1]
            for blk in range(NBLK):
                ps = mm_ps.tile([P, 512], F32, tag="mmps")
                for cc in range(NCC):
                    nc.tensor.matmul(
                        ps, lhsT=w1t_s[:, cc * H + hc * P:cc * H + (hc + 1) * P],
                        rhs=y[(s, cc)][:, blk * 512:(blk + 1) * 512],
                        start=(cc == 0), stop=False)
                # rank-1: += W1s[hc-chunk] ⊗ nmr_row  (the LN "-mu*r" term)
                nc.tensor.matmul(
                    ps, lhsT=w1s_s[:, hc * P:(hc + 1) * P],
                    rhs=nmr_row[:, blk * 512:(blk + 1) * 512],
                    start=False, stop=True)
                nc.scalar.activation(
                    out=hid[(s, hc)][:, blk * 512:(blk + 1) * 512],
                    in_=ps, func=AF.Gelu, bias=b1f_s[:, hc:hc + 1], scale=1.0)
        # GRN square+accum; y(s,0/1) are dead after pw1 reads -> reuse as
        # scratch. For the last sample (nothing left to overlap), split the
        # squares across ACT and DVE to halve the tail stall.
        gx2 = sm_p.tile([P, NHC], F32, tag="gx2", name=f"gx2_{s}{_PFX[0]}")
        gx2s[s] = gx2
        for hc in range(NHC):
            if s == BL - 1 and hc >= 2:
                sq = y[(s, 1)]
                nc.vector.tensor_mul(out=sq, in0=hid[(s, hc)], in1=hid[(s, hc)])
                nc.vector.tensor_reduce(out=gx2[:, hc:hc + 1], in_=sq,
                                        axis=mybir.AxisListType.X, op=ALU.add)
            else:
                nc.scalar.activation(out=y[(s, 0)], in_=hid[(s, hc)],
                                     func=AF.Square, accum_out=gx2[:, hc:hc + 1])

    def grn(s):
        gx2 = gx2s[s]
        gx2f = sm_p.tile([P, NHC], F32, tag="gx2f")
        nc.vector.tensor_scalar(out=gx2f, in0=gx2, scalar1=1e-30, scalar2=None,
                                op0=ALU.add)
        rg = _rsqrt(nc, sm_p, gx2f, P, NHC, "rg")
        gx = sm_p.tile([P, NHC], F32, tag="gx")
        nc.vector.tensor_mul(out=gx, in0=gx2f, in1=rg)      # gx = sqrt(gx2)
        gx_bf = sm_p.tile([P, NHC], BF16, tag="gx_bf")
        nc.vector.tensor_copy(out=gx_bf, in_=gx)
        # mean over all H=512 channels: ones-matmul -> [1,4] -> reduce
        gt_ps = st_ps.tile([1, NHC], F32, tag="stps", name=f"gt_{s}{_PFX[0]}")
        nc.tensor.matmul(gt_ps, lhsT=ones_col, rhs=gx_bf,
                         start=True, stop=True)
        g_row = sm_p.tile([1, NHC], F32, tag="g_row")
        nc.vector.tensor_copy(out=g_row, in_=gt_ps)
        tot = sm_p.tile([1, 1], F32, tag="tot")
        nc.vector.tensor_reduce(out=tot, in_=g_row, axis=mybir.AxisListType.X,
                                op=ALU.add)
        nc.vector.tensor_scalar(out=tot, in0=tot, scalar1=1.0 / H,
                                scalar2=1e-6, op0=ALU.mult, op1=ALU.add)
        rm_row = sm_p.tile([1, 1], F32, tag="rm_row")
        nc.vector.reciprocal(out=rm_row, in_=tot)
        rm_bf = sm_p.tile([1, 1], BF16, tag="rm_bf")
        nc.vector.tensor_copy(out=rm_bf, in_=rm_row)
        rm_ps = st_ps.tile([P, 1], F32, tag="stps", name=f"rm_{s}{_PFX[0]}")
        nc.tensor.matmul(rm_ps, lhsT=ones_row, rhs=rm_bf,
                         start=True, stop=True)
        rm = sm_p.tile([P, 1], F32, tag="rm")
        nc.vector.tensor_copy(out=rm, in_=rm_ps)
        a = sm_p.tile([P, NHC], F32, tag="a")
        nc.vector.tensor_scalar(out=a, in0=gx, scalar1=rm, scalar2=None,
                                op0=ALU.mult)
        nc.vector.scalar_tensor_tensor(out=a, in0=a, scalar=1.0, in1=gam_s,
                                       op0=ALU.bypass, op1=ALU.mult)
        nc.vector.tensor_scalar(out=a, in0=a, scalar1=1.0, scalar2=None,
                                op0=ALU.add)
        w2s[s] = w2s_p.tile([P, NHC * C], BF16, tag="w2s", name=f"w2s_{s}{_PFX[0]}")
        for hc in range(NHC):
            nc.vector.tensor_scalar(
                out=w2s[s][:, hc * C:(hc + 1) * C],
                in0=w2t_s[:, hc * C:(hc + 1) * C],
                scalar1=a[:, hc:hc + 1], scalar2=None, op0=ALU.mult)

    def pw2_merge(s):
        for cc in range(NCC):
            for ob_i in range(4):          # four [P, 1024] output tiles per cc
                ob = ob_p.tile([P, 1024], F32, tag="ob")
                for sub in range(2):
                    blk = ob_i * 2 + sub
                    lo = blk * 512
                    ps = mm_ps.tile([P, 512], F32, tag="mmps")
                    for hc in range(NHC):
                        nc.tensor.matmul(
                            ps,
                            lhsT=w2s[s][:, hc * C + cc * P:hc * C + (cc + 1) * P],
                            rhs=hid[(s, hc)][:, lo:lo + 512],
                            start=(hc == 0), stop=(hc == NHC - 1))
                    # out = psum + bias2 + x  (one DVE op, x read back as bf16)
                    nc.vector.scalar_tensor_tensor(
                        out=ob[:, sub * 512:(sub + 1) * 512], in0=ps,
                        scalar=b2c_s[:, cc:cc + 1],
                        in1=xb[(s, cc, blk // 4)][:, HALF + 512 * (blk % 4):
                                                  HALF + 512 * (blk % 4) + 512],
                        op0=ALU.add, op1=ALU.add)
                nc.sync.dma_start(
                    out=out_d[s, cc * P:(cc + 1) * P,
                              ob_i * 1024:(ob_i + 1) * 1024],
                    in_=ob)

    # deferred-GRN pipeline: iter s runs dw+stats(s) / grn+pw2(s-1) /
    # norm+pw1(s); gelu+square ACT tails of pw1(s) overlap dw(s+1), giving
    # the GRN chain a full iteration of slack before pw2(s) needs w2s.
    for rp in range(_REPEAT):
        _PFX[0] = f"_rp{rp}" if _REPEAT > 1 else ""
        load(0)
        for s in range(BL):
            if s + 1 < BL:
                load(s + 1)
            dw_stats(s)
            if s >= 1:
                grn(s - 1)
                pw2_merge(s - 1)
            norm(s)
            pw1(s)
        grn(BL - 1)
        pw2_merge(BL - 1)


def _prep_inputs(inputs):
    x = np.ascontiguousarray(np.asarray(inputs["x"], np.float32))
    dw_w = np.asarray(inputs["dw_w"], np.float32)      # (C,1,K)
    dw_b = np.asarray(inputs["dw_b"], np.float32)
    ln_w = np.asarray(inputs["ln_w"], np.float32)
    ln_b = np.asarray(inputs["ln_b"], np.float32)
    pw1_w = np.asarray(inputs["pw1_w"], np.float32)    # (H,C)
    pw1_b = np.asarray(inputs["pw1_b"], np.float32)
    gg = np.asarray(inputs["grn_gamma"], np.float32)
    gb = np.asarray(inputs["grn_beta"], np.float32)
    pw2_w = np.asarray(inputs["pw2_w"], np.float32)    # (C,H)
    pw2_b = np.asarray(inputs["pw2_b"], np.float32)

    ident = np.eye(P, dtype=BF)
    dww = np.zeros((P, K * NCC), np.float32)
    for k in range(K):
        for cc in range(NCC):
            dww[:, k * NCC + cc] = dw_w[cc * P:(cc + 1) * P, 0, k]
    dwb = dw_b.reshape(NCC, P).T.copy()

    w1f = pw1_w * ln_w[None, :]                        # (H,C)
    w1t = np.zeros((P, NCC * H), BF)
    for cc in range(NCC):
        for hc in range(NHC):
            w1t[:, cc * H + hc * P:cc * H + (hc + 1) * P] = \
                w1f[hc * P:(hc + 1) * P, cc * P:(cc + 1) * P].T.astype(BF)
    b1f = (pw1_b + pw1_w @ ln_b).reshape(NHC, P).T.copy()
    w1s = w1f.sum(axis=1).astype(BF).reshape(1, H)

    w2t = np.zeros((P, NHC * C), BF)
    for hc in range(NHC):
        w2t[:, hc * C:(hc + 1) * C] = \
            pw2_w[:, hc * P:(hc + 1) * P].T.astype(BF)
    gam = gg.reshape(NHC, P).T.copy()
    b2c = (pw2_b + pw2_w @ gb).reshape(NCC, P).T.copy()

    onescol = np.ones((P, 1), BF)
    w1s_blk = np.zeros((P, H), BF)
    w1s_blk[0, :] = w1s[0, :]
    onesrow_blk = np.zeros((P, P), BF)
    onesrow_blk[0, :] = 1.0
    cpack = np.concatenate([
        dww.view(np.uint8), dwb.view(np.uint8), b1f.view(np.uint8),
        gam.view(np.uint8), b2c.view(np.uint8), ident.view(np.uint8),
        w1t.view(np.uint8), w2t.view(np.uint8), onescol.view(np.uint8),
        w1s_blk.view(np.uint8), onesrow_blk.view(np.uint8),
        np.zeros((P, 2), np.uint8)], axis=1)
    assert cpack.shape == (P, 5756), cpack.shape
    common = {"cpack": np.ascontiguousarray(cpack)}
    in_maps = []
    for i in range(NCORES):
        m = dict(common)
        m["x"] = np.ascontiguousarray(x[i * BL:(i + 1) * BL])
        in_maps.append(m)
    return in_maps


def kernel(**inputs):
    if "nc" not in _CACHE:
        _CACHE["nc"] = _build()
    nc = _CACHE["nc"]
    in_maps = _prep_inputs(inputs)
    res = run_bass_kernel_spmd(nc, in_maps, core_ids=list(range(NCORES)),
                               **_CACHE.get("run_kwargs", {}))
    _CACHE["last_result"] = res
    out = np.concatenate([res.results[i]["out"] for i in range(NCORES)], axis=0)
    return out

